# revision 20
# baseline (speedup 1.0000x reference)
"""RBF kernel attention (nn_KernelAttention) on 8 Trainium2 NeuronCores.

reference math (per batch b):
    dist2[i,j] = ||x_i||^2 + ||x_j||^2 - 2 x_i.x_j
    attn = softmax(-gamma * max(dist2, 0), axis=j)
    out  = attn @ x

Two device kernels:

1. FAST (block-diagonal) path.  For an RBF kernel the logit of pair
   (i, j) is -gamma * dist2(i,j) <= 0 while the diagonal logit is always
   exactly 0, so whenever gamma*dist2 >= ~30 for every off-diagonal pair
   outside the 128-row diagonal blocks, those pairs contribute
   < 4096*e^-30 ~ 4e-10 of each row's softmax mass and the attention
   matrix is block-diagonal to (far beyond) f32 precision.  The host
   CERTIFIES this per input with exact bounds before choosing this path:
     * within each 128-row block: exact pairwise distances (cheap gram),
     * across blocks: dist2(i,j) >= ||U^T x_i - U^T x_j||^2 for any
       orthonormal U (columns), checked with a 128-dim projection,
     * plus bf16 rounding-slop / overflow guards for the device math.
   The device kernel then runs flash-attention-style row blocks over the
   certified mask: QK^T gram, exp, PV and row-sum normalization for the
   16 diagonal 128x128 blocks only.

2. DENSE fallback: full 4096-key attention per query (the original
   kernel), used whenever certification fails.

Sharding (both paths): core c handles batch c//2, query half c%2 (2048
queries).  No collectives; host concatenates.
"""

import sys

if "/opt/trn_rl_repo" not in sys.path:
    sys.path.insert(0, "/opt/trn_rl_repo")

from contextlib import ExitStack

import numpy as np

import concourse.bass as bass
import concourse.mybir as mybir
import concourse.tile as tile
from concourse import bacc
from concourse.bass_utils import run_bass_kernel_spmd
from concourse.masks import make_identity

F32 = mybir.dt.float32
BF16 = mybir.dt.bfloat16
FP8 = mybir.dt.float8e4
AF = mybir.ActivationFunctionType

FP8_QK = True   # dense path: fp8 DoubleRow for the Q@K^T gram matmul
XBAR_T = False  # fast path: build x^T via DMA crossbar instead of PE
CAST_DMA = True  # fast path: gpsimd software-DGE DMAs cast f32->bf16 on load

B, S, E = 4, 4096, 1024
NCORES = 8
P = 128                 # partitions
SQ = S // 2             # queries per core
NB = SQ // P            # 16 diagonal blocks per core (fast path)
NKB = S // P            # 32 key blocks (dense path)
NKB_SELF = SQ // P      # 16 key blocks coming from x_self
NEC = E // P            # 8 contraction chunks for Q@K^T
QB = 512                # dense: query free-dim tile for QK / exp
NQB = SQ // QB          # 4
NQS = QB // P           # 4 query subtiles per query block
EH = 512                # PV free-dim half (PSUM bank limit)


# --------------------------------------------------------------------------
# fast path: certified block-diagonal attention
# --------------------------------------------------------------------------

def _build_fast_body(ctx: ExitStack, tc: tile.TileContext, gamma: float,
                     x_d, out_d):
    nc = tc.nc
    g = float(gamma)

    const = ctx.enter_context(tc.tile_pool(name="const", bufs=1))
    xin = ctx.enter_context(tc.tile_pool(name="xin", bufs=1))
    sqd = ctx.enter_context(tc.tile_pool(name="sqd", bufs=2))
    vpool = ctx.enter_context(tc.tile_pool(name="vpool", bufs=6))
    xtp = ctx.enter_context(tc.tile_pool(name="xtp", bufs=3))
    ptp = ctx.enter_context(tc.tile_pool(name="ptp", bufs=3))
    opool = ctx.enter_context(tc.tile_pool(name="opool", bufs=3))
    small = ctx.enter_context(tc.tile_pool(name="small", bufs=4))

    # const setup first: make_identity is two tiny gpsimd ops that must
    # not queue behind the gpsimd software-DGE loads below (PE transposes
    # wait on the identity)
    ident = const.tile([P, P], BF16, name="ident", tag="ident")
    make_identity(nc, ident)
    onesrow = const.tile([1, P], BF16, name="onesrow", tag="onesrow")
    nc.vector.memset(onesrow, 1.0)
    sq_all = const.tile([P, NB], F32, name="sq_all", tag="sq_all")
    biasK = const.tile([P, NB], F32, name="biasK", tag="biasK")

    # stage all 16 input block DMAs up front.  Head blocks go through
    # fast HWDGE f32 loads (+DVE cast, DVE is idle early); the rest
    # through gpsimd software-DGE which converts f32->bf16 in flight.
    HYB = 4 if CAST_DMA else NB
    Vs = [None] * NB
    xst = [None] * NB
    for kb in range(NB):
        if kb < HYB:
            t = xin.tile([P, E], F32, name=f"xst{kb}", tag=f"xst{kb}")
            nc.sync.dma_start(out=t, in_=x_d[kb * P:(kb + 1) * P, :])
            xst[kb] = t
        else:
            V = xin.tile([P, E], BF16, name=f"V{kb}", tag=f"V{kb}")
            nc.gpsimd.dma_start(out=V, in_=x_d[kb * P:(kb + 1) * P, :])
            Vs[kb] = V

    # PSUM: 2 (G) + 2 (transpose scratch) + 4 (PV out) = 8 banks
    qk_ps = ctx.enter_context(tc.tile_pool(name="qk_ps", bufs=2, space="PSUM"))
    tr_ps = ctx.enter_context(tc.tile_pool(name="tr_ps", bufs=2, space="PSUM"))
    out_ps = ctx.enter_context(tc.tile_pool(name="out_ps", bufs=2, space="PSUM"))

    sq_scs = [None] * NB
    xTs = [None] * NB
    sqrows = [None] * NB

    def stage1(kb):
        # bf16 V, ||x||^2 via ACT Square accumulate over the bf16 values
        # (2x ACT rate, and consistent with the bf16 gram), bias columns
        if Vs[kb] is not None:
            V = Vs[kb]
        else:
            V = vpool.tile([P, E], BF16, name="V", tag="V")
            nc.vector.tensor_copy(V, xst[kb])
            Vs[kb] = V
        sqt = sqd.tile([P, E], BF16, name="sqt", tag="sqt")
        nc.scalar.activation(sqt, V, AF.Square,
                             accum_out=sq_all[:, kb:kb + 1])
        sq_sc = small.tile([P, 1], BF16, name="sq_sc", tag="sq_sc")
        nc.vector.tensor_scalar_mul(sq_sc, sq_all[:, kb:kb + 1], -0.5)
        # bias built from the SAME bf16 rounding as the free-dim term so
        # the logit matrix stays (nearly) symmetric
        nc.vector.tensor_scalar_mul(biasK[:, kb:kb + 1], sq_sc, 2.0 * g)
        Vs[kb] = V
        sq_scs[kb] = sq_sc

    def stage2a(kb):
        # x^T chunks: DMA crossbar (one transfer) or PE identity matmuls
        V = Vs[kb]
        xT = xtp.tile([P, NEC, P], BF16, name="xT", tag="xT")
        if XBAR_T:
            nc.sync.dma_start_transpose(xT, V)
            srp = tr_ps.tile([P, P], BF16, name="srp", tag="srp")
            nc.tensor.transpose(srp[0:1, :], sq_scs[kb], ident)
            sqrow = small.tile([1, P], BF16, name="sqrow", tag="sqrow")
            nc.vector.tensor_copy(sqrow, srp[0:1, :])
        else:
            # quad-packed: 4 transposes per PSUM tile, one DVE drain each
            for c in range(2):
                trp = tr_ps.tile([P, 5 * P], BF16, name="trp", tag="trp")
                for i in range(4):
                    nc.tensor.transpose(
                        trp[:, i * P:(i + 1) * P],
                        V[:, (4 * c + i) * P:(4 * c + i + 1) * P], ident)
                if c == 1:
                    # piggyback the sq-row transpose on the last trp tile
                    nc.tensor.transpose(trp[0:1, 4 * P:5 * P],
                                        sq_scs[kb], ident)
                nc.vector.tensor_copy(
                    xT[:, 4 * c:4 * c + 4, :],
                    trp[:, 0:4 * P].rearrange("p (i k) -> p i k", i=4))
                if c == 1:
                    sqrow = small.tile([1, P], BF16, name="sqrow",
                                       tag="sqrow")
                    nc.vector.tensor_copy(sqrow, trp[0:1, 4 * P:5 * P])
        xTs[kb] = xT
        sqrows[kb] = sqrow

    def stage2b(kb):
        # G = X X^T (+ rank-1 -0.5*sq_q row);
        # P^T = exp(2g*(G - .5 sq_q - .5 sq_k))
        xT = xTs[kb]
        qkp = qk_ps.tile([P, P], F32, name="qkp", tag="qkp")
        for c in range(NEC):
            nc.tensor.matmul(qkp, lhsT=xT[:, c, :], rhs=xT[:, c, :],
                             start=(c == 0), stop=False)
        nc.tensor.matmul(qkp, lhsT=onesrow, rhs=sqrows[kb],
                         start=False, stop=True)
        pt = ptp.tile([P, P], BF16, name="pt", tag="pt")
        nc.scalar.activation(pt, qkp, AF.Exp, bias=biasK[:, kb:kb + 1],
                             scale=2.0 * g)
        return pt

    def stage3(kb, pt):
        # row-sum from the bf16 weights (so numerator/denominator share
        # one rounding), out rows = (P^T)^T @ V / rowsum; normalize
        # alternates ACT/DVE
        V = Vs[kb]
        sumc = small.tile([P, 1], F32, name="sumc", tag="sumc")
        nc.vector.reduce_sum(out=sumc, in_=pt, axis=mybir.AxisListType.X)
        po = out_ps.tile([P, E], F32, name="po", tag="po")
        nc.tensor.matmul(po[:, 0:EH], lhsT=pt, rhs=V[:, 0:EH])
        nc.tensor.matmul(po[:, EH:E], lhsT=pt, rhs=V[:, EH:E])
        rc = small.tile([P, 1], F32, name="rc", tag="rc")
        nc.vector.reciprocal(rc, sumc)
        ot = opool.tile([P, E], F32, name="ot", tag="ot")
        if kb % 2 == 0:
            nc.scalar.activation(ot, po, AF.Copy, scale=rc)
        else:
            nc.vector.tensor_scalar_mul(ot, po, rc)
        nc.sync.dma_start(out=out_d[kb * P:(kb + 1) * P, :], in_=ot)

    # software-pipelined emission, two stages ahead: stage1 (cast/sq) at
    # kb+2, x^T production at kb+1 (covers DMA-crossbar latency), G/exp
    # at kb, PV/store at kb-1
    stage1(0)
    stage1(1)
    stage2a(0)
    pend = None
    for kb in range(NB):
        if kb + 2 < NB:
            stage1(kb + 2)
        if kb + 1 < NB:
            stage2a(kb + 1)
        pt = stage2b(kb)
        if pend is not None:
            stage3(kb - 1, pend)
        pend = pt
    stage3(NB - 1, pend)


def build_fast_module(gamma: float):
    nc = bacc.Bacc("TRN2", target_bir_lowering=False, debug=False)
    x_d = nc.dram_tensor("xq", [SQ, E], F32, kind="ExternalInput")
    out_d = nc.dram_tensor("out", [SQ, E], F32, kind="ExternalOutput")
    with tile.TileContext(nc) as tc, ExitStack() as ctx:
        _build_fast_body(ctx, tc, gamma, x_d, out_d)
    nc.compile()
    return nc


# --------------------------------------------------------------------------
# host-side certification of the block-diagonal mask (exact bounds)
# --------------------------------------------------------------------------

def _screen_fast_ok(x: np.ndarray, gamma: float) -> bool:
    """True iff the fast path is certified correct for this input.

    Guards (g = gamma):
      (a) g > 0 and fp8/bf16 rounding slop of the device logits stays
          < ~55 in exp-argument units (no f32 overflow, no weight
          distortion the margins below can't absorb),
      (b) exact within-block off-diagonal dist2 >= 120/g (threshold 40
          for negligibility + slop headroom for the fp8 gram),
      (c) projected (certified lower-bound) cross-block dist2 >= 31/g.
    """
    g = float(gamma)
    if not np.isfinite(g) or g <= 0.0:
        return False
    Bx, Sx, Ex = x.shape
    sq = np.einsum('bse,bse->bs', x, x).astype(np.float64)
    smax = float(sq.max())
    # device-side exp-argument slop: fp8 gram err tail ~0.012*smax,
    # bf16 sq-row err ~0.001*smax -> arg err <~ 2g*0.013*smax
    if 0.026 * g * smax > 55.0:
        return False
    # (b) exact within-block pairwise distances
    nblk = Sx // P
    xb = np.ascontiguousarray(x.reshape(Bx, nblk, P, Ex))
    Gb = np.einsum('bkie,bkje->bkij', xb, xb)
    sb = sq.reshape(Bx, nblk, P)
    d2b = sb[:, :, :, None] + sb[:, :, None, :] - 2.0 * Gb
    ii = np.arange(P)
    d2b[:, :, ii, ii] = np.inf
    if float(d2b.min()) < 120.0 / g:
        return False
    # (c) cross-block: dist2 >= ||proj(x_i) - proj(x_j)||^2 for
    # orthonormal projection columns (exact bound); 128 dims
    rng = np.random.default_rng(0xA55E77)
    U, _ = np.linalg.qr(rng.standard_normal((Ex, 128)))
    U = np.ascontiguousarray(U.astype(np.float32))
    for b in range(Bx):
        y = x[b] @ U
        sy = np.einsum('se,se->s', y, y)
        Gy = y @ y.T
        d2 = sy[:, None] + sy[None, :] - 2.0 * Gy
        v = d2.reshape(nblk, P, nblk, P)
        for k in range(nblk):
            v[k, :, k, :] = np.inf
        # +1.0 absorbs f32 rounding in this host-side bound computation
        if float(d2.min()) < 31.0 / g + 1.0:
            return False
    return True


# --------------------------------------------------------------------------
# dense fallback (original kernel, unchanged)
# --------------------------------------------------------------------------

def _build_dense_body(ctx: ExitStack, tc: tile.TileContext, gamma: float,
                      xs_d, xo_d, out_d, sqq_d):
    nc = tc.nc

    const = ctx.enter_context(tc.tile_pool(name="const", bufs=1))
    stage = ctx.enter_context(tc.tile_pool(name="stage", bufs=4))
    tpool = ctx.enter_context(tc.tile_pool(name="tpool", bufs=3))
    opool = ctx.enter_context(tc.tile_pool(name="opool", bufs=2))
    small = ctx.enter_context(tc.tile_pool(name="small", bufs=2))
    ptp = ctx.enter_context(tc.tile_pool(name="ptp", bufs=1))

    # ---- persistent SBUF tiles ----
    if FP8_QK:
        # [256-e-chunk][e_part, pair, k]; logical e = 256*c + 128*i + p
        xT8 = [const.tile([P, 2, S], FP8, name=f"xT8{c}", tag=f"xT8{c}")
               for c in range(NEC // 2)]
    else:
        xT = [const.tile([P, S], BF16, name=f"xT{e}", tag=f"xT{e}")
              for e in range(NEC)]                   # [E-chunk][e_part, k]
    V = [const.tile([P, E], BF16, name=f"V{kb}", tag=f"V{kb}")
         for kb in range(NKB)]                       # [k-block][k_part, e]
    sq_all = const.tile([P, NKB], F32, name="sq_all", tag="sq_all")
    biasK = const.tile([P, NKB], F32, name="biasK", tag="biasK")
    sqq_sc = const.tile([P, NKB_SELF], BF16, name="sqq_sc", tag="sqq_sc")
    bcastQ = const.tile([P, SQ], BF16, name="bcastQ", tag="bcastQ")
    ones = const.tile([P, 1], BF16, name="ones", tag="ones")
    nc.vector.memset(ones, 1.0)
    ident = const.tile([P, P], BF16, name="ident", tag="ident")
    make_identity(nc, ident)

    # ---- prologue: load x, compute ||x||^2, cast to bf16, build x^T ----
    with tc.tile_pool(name="sq_ps", bufs=2, space="PSUM") as sq_ps, \
         tc.tile_pool(name="tr_ps", bufs=4, space="PSUM") as tr_ps:
        for kb in range(NKB):
            src = xs_d if kb < NKB_SELF else xo_d
            r0 = (kb % NKB_SELF) * P
            xst = stage.tile([P, E], F32, name="xst", tag="xst")
            nc.sync.dma_start(out=xst, in_=src[r0:r0 + P, :])
            nc.gpsimd.tensor_copy(V[kb], xst)        # f32 -> bf16 cast
            sqt = sq_ps.tile([P, E], F32, name="sqt", tag="sqt")
            nc.scalar.activation(sqt, xst, AF.Square,
                                 accum_out=sq_all[:, kb:kb + 1])
            if FP8_QK:
                for c in range(NEC // 2):
                    trp = tr_ps.tile([P, 2 * P], BF16, name="trp", tag="trp")
                    for i in range(2):
                        nc.tensor.transpose(
                            trp[:, i * P:(i + 1) * P],
                            V[kb][:, (2 * c + i) * P:(2 * c + i + 1) * P],
                            ident)
                    nc.vector.tensor_copy(
                        xT8[c][:, :, kb * P:(kb + 1) * P],
                        trp.rearrange("p (i k) -> p i k", i=2))
            else:
                for e in range(NEC):
                    trp = tr_ps.tile([P, P], BF16, name="trp", tag="trp")
                    nc.tensor.transpose(trp, V[kb][:, e * P:(e + 1) * P],
                                        ident)
                    nc.vector.tensor_copy(xT[e][:, kb * P:(kb + 1) * P], trp)
            if kb == NKB_SELF - 1:
                # self-half stats ready: unblock exp biases + bcastQ early
                nc.vector.tensor_scalar_mul(
                    biasK[:, :NKB_SELF], sq_all[:, :NKB_SELF], -gamma)
                nc.vector.tensor_scalar_mul(
                    sqq_sc, sq_all[:, :NKB_SELF], -0.5)
                nc.sync.dma_start(
                    out=sqq_d[:].rearrange("(c p) -> p c", p=P), in_=sqq_sc)
                s_ap = sqq_d[:]
                bq_src = bass.AP(tensor=s_ap.tensor, offset=s_ap.offset,
                                 ap=[[0, P]] + list(s_ap.ap))
                nc.sync.dma_start(out=bcastQ, in_=bq_src)

    nc.vector.tensor_scalar_mul(biasK[:, NKB_SELF:], sq_all[:, NKB_SELF:],
                                -gamma)

    # ---- main loop: PSUM pools (8 banks total: 2 + 4 + 2) ----
    qk_ps = ctx.enter_context(tc.tile_pool(name="qk_ps", bufs=3, space="PSUM"))
    out_ps = ctx.enter_context(tc.tile_pool(name="out_ps", bufs=2, space="PSUM"))
    s_ps = ctx.enter_context(tc.tile_pool(name="s_ps", bufs=1, space="PSUM"))

    for qb in range(NQB):
        q0 = qb * QB
        # Phase A: P^T[k, q0:q0+QB] for all 32 key blocks
        pts = []
        for kb in range(NKB):
            qkp = qk_ps.tile([P, QB], F32, name="qkp", tag="qkp")
            if FP8_QK:
                for c in range(NEC // 2):
                    nc.tensor.matmul(qkp,
                                     lhsT=xT8[c][:, :, kb * P:(kb + 1) * P],
                                     rhs=xT8[c][:, :, q0:q0 + QB],
                                     start=(c == 0), stop=(c == NEC // 2 - 1),
                                     perf_mode=mybir.MatmulPerfMode.DoubleRow)
            else:
                for e in range(NEC):
                    nc.tensor.matmul(qkp,
                                     lhsT=xT[e][:, kb * P:(kb + 1) * P],
                                     rhs=xT[e][:, q0:q0 + QB],
                                     start=(e == 0), stop=(e == NEC - 1))
            tt = tpool.tile([P, QB], F32, name="tt", tag="tt")
            nc.vector.tensor_add(tt, qkp, bcastQ[:, q0:q0 + QB])
            pt = ptp.tile([P, QB], BF16, name=f"pt{kb}", tag=f"pt{kb}")
            nc.scalar.activation(pt, tt, AF.Exp,
                                 bias=biasK[:, kb:kb + 1], scale=2.0 * gamma)
            pts.append(pt)
        # Phase B: out[q, :] = (P^T)^T @ V, row-sum via ones column
        for qs in range(NQS):
            po = out_ps.tile([P, E], F32, name="po", tag="po")
            sp = s_ps.tile([P, 1], F32, name="sp", tag="sp")
            for kb in range(NKB):
                lw = pts[kb][:, qs * P:(qs + 1) * P]
                nc.tensor.matmul(po[:, 0:EH], lhsT=lw, rhs=V[kb][:, 0:EH],
                                 start=(kb == 0), stop=(kb == NKB - 1))
                nc.tensor.matmul(po[:, EH:E], lhsT=lw, rhs=V[kb][:, EH:E],
                                 start=(kb == 0), stop=(kb == NKB - 1))
                nc.tensor.matmul(sp, lhsT=lw, rhs=ones,
                                 start=(kb == 0), stop=(kb == NKB - 1))
            rc = small.tile([P, 1], F32, name="rc", tag="rc")
            nc.vector.reciprocal(rc, sp)
            ot = opool.tile([P, E], F32, name="ot", tag="ot")
            nc.vector.tensor_scalar_mul(ot, po, rc)
            nc.sync.dma_start(out=out_d[q0 + qs * P:q0 + (qs + 1) * P, :],
                              in_=ot)


def build_dense_module(gamma: float):
    nc = bacc.Bacc("TRN2", target_bir_lowering=False, debug=False)
    xs_d = nc.dram_tensor("x_self", [SQ, E], F32, kind="ExternalInput")
    xo_d = nc.dram_tensor("x_other", [SQ, E], F32, kind="ExternalInput")
    out_d = nc.dram_tensor("out", [SQ, E], F32, kind="ExternalOutput")
    sqq_d = nc.dram_tensor("sqq_scratch", [SQ], BF16)
    with tile.TileContext(nc) as tc, ExitStack() as ctx:
        _build_dense_body(ctx, tc, gamma, xs_d, xo_d, out_d, sqq_d)
    nc.compile()
    return nc


_CACHE: dict = {}


def _get_module(gamma: float, kind: str = "dense"):
    key = (kind, gamma)
    if key not in _CACHE:
        _CACHE[key] = (build_fast_module(gamma) if kind == "fast"
                       else build_dense_module(gamma))
    return _CACHE[key]


def kernel(x, gamma):
    x = np.ascontiguousarray(np.asarray(x, dtype=np.float32))
    g = float(np.asarray(gamma))
    if _screen_fast_ok(x, g):
        nc = _get_module(g, "fast")
        in_maps = []
        for c in range(NCORES):
            b, h = divmod(c, 2)
            in_maps.append({"xq": np.ascontiguousarray(
                x[b, h * SQ:(h + 1) * SQ])})
        res = run_bass_kernel_spmd(nc, in_maps, list(range(NCORES))).results
    else:
        nc = _get_module(g, "dense")
        in_maps = []
        for c in range(NCORES):
            b, h = divmod(c, 2)
            xs = np.ascontiguousarray(x[b, h * SQ:(h + 1) * SQ])
            xo = np.ascontiguousarray(x[b, (1 - h) * SQ:(2 - h) * SQ])
            in_maps.append({"x_self": xs, "x_other": xo})
        res = run_bass_kernel_spmd(nc, in_maps, list(range(NCORES))).results
    out = np.empty((B, S, E), np.float32)
    for c in range(NCORES):
        b, h = divmod(c, 2)
        out[b, h * SQ:(h + 1) * SQ] = res[c]["out"]
    return out


if __name__ == "__main__":
    xs = np.random.randn(B, S, E).astype(np.float32)
    o = kernel(xs, np.float32(1.0))
    print("ran", o.shape, o.dtype)


# revision 21
# speedup vs baseline: 1.0897x; 1.0897x over previous
"""RBF kernel attention (nn_KernelAttention) on 8 Trainium2 NeuronCores.

reference math (per batch b):
    dist2[i,j] = ||x_i||^2 + ||x_j||^2 - 2 x_i.x_j
    attn = softmax(-gamma * max(dist2, 0), axis=j)
    out  = attn @ x

Two device kernels:

1. FAST (block-diagonal) path.  For an RBF kernel the logit of pair
   (i, j) is -gamma * dist2(i,j) <= 0 while the diagonal logit is always
   exactly 0, so whenever gamma*dist2 >= ~30 for every off-diagonal pair
   outside the 128-row diagonal blocks, those pairs contribute
   < 4096*e^-30 ~ 4e-10 of each row's softmax mass and the attention
   matrix is block-diagonal to (far beyond) f32 precision.  The host
   CERTIFIES this per input with exact bounds before choosing this path:
     * within each 128-row block: exact pairwise distances (cheap gram),
     * across blocks: dist2(i,j) >= ||U^T x_i - U^T x_j||^2 for any
       orthonormal U (columns), checked with a 128-dim projection,
     * plus bf16 rounding-slop / overflow guards for the device math.
   The device kernel then runs flash-attention-style row blocks over the
   certified mask: QK^T gram, exp, PV and row-sum normalization for the
   16 diagonal 128x128 blocks only.

2. DENSE fallback: full 4096-key attention per query (the original
   kernel), used whenever certification fails.

Sharding (both paths): core c handles batch c//2, query half c%2 (2048
queries).  No collectives; host concatenates.
"""

import sys

if "/opt/trn_rl_repo" not in sys.path:
    sys.path.insert(0, "/opt/trn_rl_repo")

from contextlib import ExitStack

import numpy as np

import concourse.bass as bass
import concourse.mybir as mybir
import concourse.tile as tile
from concourse import bacc
from concourse.bass_utils import run_bass_kernel_spmd
from concourse.masks import make_identity

F32 = mybir.dt.float32
BF16 = mybir.dt.bfloat16
FP8 = mybir.dt.float8e4
AF = mybir.ActivationFunctionType

FP8_QK = True   # dense path: fp8 DoubleRow for the Q@K^T gram matmul
XBAR_T = False  # fast path: build x^T via DMA crossbar instead of PE
CAST_DMA = True  # fast path: gpsimd software-DGE DMAs cast f32->bf16 on load

B, S, E = 4, 4096, 1024
NCORES = 8
P = 128                 # partitions
SQ = S // 2             # queries per core
NB = SQ // P            # 16 diagonal blocks per core (fast path)
NKB = S // P            # 32 key blocks (dense path)
NKB_SELF = SQ // P      # 16 key blocks coming from x_self
NEC = E // P            # 8 contraction chunks for Q@K^T
QB = 512                # dense: query free-dim tile for QK / exp
NQB = SQ // QB          # 4
NQS = QB // P           # 4 query subtiles per query block
EH = 512                # PV free-dim half (PSUM bank limit)


# --------------------------------------------------------------------------
# fast path: certified block-diagonal attention
# --------------------------------------------------------------------------

def _build_fast_body(ctx: ExitStack, tc: tile.TileContext, gamma: float,
                     x_d, out_d):
    nc = tc.nc
    g = float(gamma)

    const = ctx.enter_context(tc.tile_pool(name="const", bufs=1))
    xin = ctx.enter_context(tc.tile_pool(name="xin", bufs=1))
    sqd = ctx.enter_context(tc.tile_pool(name="sqd", bufs=2))
    vpool = ctx.enter_context(tc.tile_pool(name="vpool", bufs=6))
    xtp = ctx.enter_context(tc.tile_pool(name="xtp", bufs=3))
    ptp = ctx.enter_context(tc.tile_pool(name="ptp", bufs=3))
    opool = ctx.enter_context(tc.tile_pool(name="opool", bufs=3))
    small = ctx.enter_context(tc.tile_pool(name="small", bufs=4))

    # const setup first: make_identity is two tiny gpsimd ops that must
    # not queue behind the gpsimd software-DGE loads below (PE transposes
    # wait on the identity)
    ident = const.tile([P, P], BF16, name="ident", tag="ident")
    make_identity(nc, ident)
    onesrow = const.tile([1, P], BF16, name="onesrow", tag="onesrow")
    nc.vector.memset(onesrow, 1.0)
    sq_all = const.tile([P, NB], F32, name="sq_all", tag="sq_all")
    biasK = const.tile([P, NB], F32, name="biasK", tag="biasK")

    # stage all 16 input block DMAs up front.  Head blocks go through
    # fast HWDGE f32 loads (+DVE cast, DVE is idle early); the rest
    # through gpsimd software-DGE which converts f32->bf16 in flight.
    HYB = 8 if CAST_DMA else NB
    Vs = [None] * NB
    xst = [None] * NB
    for kb in range(NB):
        if kb < HYB:
            t = xin.tile([P, E], F32, name=f"xst{kb}", tag=f"xst{kb}")
            nc.sync.dma_start(out=t, in_=x_d[kb * P:(kb + 1) * P, :])
            xst[kb] = t
        else:
            V = xin.tile([P, E], BF16, name=f"V{kb}", tag=f"V{kb}")
            nc.gpsimd.dma_start(out=V, in_=x_d[kb * P:(kb + 1) * P, :])
            Vs[kb] = V

    # PSUM: 2 (G) + 2 (transpose scratch) + 4 (PV out) = 8 banks
    qk_ps = ctx.enter_context(tc.tile_pool(name="qk_ps", bufs=2, space="PSUM"))
    tr_ps = ctx.enter_context(tc.tile_pool(name="tr_ps", bufs=2, space="PSUM"))
    out_ps = ctx.enter_context(tc.tile_pool(name="out_ps", bufs=2, space="PSUM"))

    sq_scs = [None] * NB
    xTs = [None] * NB
    sqrows = [None] * NB

    def stage1(kb):
        # bf16 V, ||x||^2 via ACT Square accumulate over the bf16 values
        # (2x ACT rate, and consistent with the bf16 gram), bias columns
        if Vs[kb] is not None:
            V = Vs[kb]
        else:
            V = vpool.tile([P, E], BF16, name="V", tag="V")
            nc.vector.tensor_copy(V, xst[kb])
            Vs[kb] = V
        sqt = sqd.tile([P, E], BF16, name="sqt", tag="sqt")
        nc.scalar.activation(sqt, V, AF.Square,
                             accum_out=sq_all[:, kb:kb + 1])
        sq_sc = small.tile([P, 1], BF16, name="sq_sc", tag="sq_sc")
        nc.vector.tensor_scalar_mul(sq_sc, sq_all[:, kb:kb + 1], -0.5)
        # bias built from the SAME bf16 rounding as the free-dim term so
        # the logit matrix stays (nearly) symmetric
        nc.vector.tensor_scalar_mul(biasK[:, kb:kb + 1], sq_sc, 2.0 * g)
        Vs[kb] = V
        sq_scs[kb] = sq_sc

    def stage2a(kb):
        # x^T chunks: DMA crossbar (one transfer) or PE identity matmuls
        V = Vs[kb]
        xT = xtp.tile([P, NEC, P], BF16, name="xT", tag="xT")
        if XBAR_T:
            nc.sync.dma_start_transpose(xT, V)
            srp = tr_ps.tile([P, P], BF16, name="srp", tag="srp")
            nc.tensor.transpose(srp[0:1, :], sq_scs[kb], ident)
            sqrow = small.tile([1, P], BF16, name="sqrow", tag="sqrow")
            nc.vector.tensor_copy(sqrow, srp[0:1, :])
        else:
            # quad-packed: 4 transposes per PSUM tile, one DVE drain each
            for c in range(2):
                trp = tr_ps.tile([P, 5 * P], BF16, name="trp", tag="trp")
                for i in range(4):
                    nc.tensor.transpose(
                        trp[:, i * P:(i + 1) * P],
                        V[:, (4 * c + i) * P:(4 * c + i + 1) * P], ident)
                if c == 1:
                    # piggyback the sq-row transpose on the last trp tile
                    nc.tensor.transpose(trp[0:1, 4 * P:5 * P],
                                        sq_scs[kb], ident)
                nc.vector.tensor_copy(
                    xT[:, 4 * c:4 * c + 4, :],
                    trp[:, 0:4 * P].rearrange("p (i k) -> p i k", i=4))
                if c == 1:
                    sqrow = small.tile([1, P], BF16, name="sqrow",
                                       tag="sqrow")
                    nc.vector.tensor_copy(sqrow, trp[0:1, 4 * P:5 * P])
        xTs[kb] = xT
        sqrows[kb] = sqrow

    def stage2b(kb):
        # G = X X^T (+ rank-1 -0.5*sq_q row);
        # P^T = exp(2g*(G - .5 sq_q - .5 sq_k))
        xT = xTs[kb]
        qkp = qk_ps.tile([P, P], F32, name="qkp", tag="qkp")
        for c in range(NEC):
            nc.tensor.matmul(qkp, lhsT=xT[:, c, :], rhs=xT[:, c, :],
                             start=(c == 0), stop=False)
        nc.tensor.matmul(qkp, lhsT=onesrow, rhs=sqrows[kb],
                         start=False, stop=True)
        pt = ptp.tile([P, P], BF16, name="pt", tag="pt")
        nc.scalar.activation(pt, qkp, AF.Exp, bias=biasK[:, kb:kb + 1],
                             scale=2.0 * g)
        return pt

    def stage3(kb, pt):
        # row-sum from the bf16 weights (so numerator/denominator share
        # one rounding), out rows = (P^T)^T @ V / rowsum; normalize
        # alternates ACT/DVE
        V = Vs[kb]
        sumc = small.tile([P, 1], F32, name="sumc", tag="sumc")
        nc.vector.reduce_sum(out=sumc, in_=pt, axis=mybir.AxisListType.X)
        po = out_ps.tile([P, E], F32, name="po", tag="po")
        nc.tensor.matmul(po[:, 0:EH], lhsT=pt, rhs=V[:, 0:EH])
        nc.tensor.matmul(po[:, EH:E], lhsT=pt, rhs=V[:, EH:E])
        rc = small.tile([P, 1], F32, name="rc", tag="rc")
        nc.vector.reciprocal(rc, sumc)
        ot = opool.tile([P, E], F32, name="ot", tag="ot")
        if kb % 2 == 0:
            nc.scalar.activation(ot, po, AF.Copy, scale=rc)
        else:
            nc.vector.tensor_scalar_mul(ot, po, rc)
        nc.sync.dma_start(out=out_d[kb * P:(kb + 1) * P, :], in_=ot)

    # software-pipelined emission, two stages ahead: stage1 (cast/sq) at
    # kb+2, x^T production at kb+1 (covers DMA-crossbar latency), G/exp
    # at kb, PV/store at kb-1
    stage1(0)
    stage1(1)
    stage2a(0)
    pend = None
    for kb in range(NB):
        if kb + 2 < NB:
            stage1(kb + 2)
        if kb + 1 < NB:
            stage2a(kb + 1)
        pt = stage2b(kb)
        if pend is not None:
            stage3(kb - 1, pend)
        pend = pt
    stage3(NB - 1, pend)


def build_fast_module(gamma: float):
    nc = bacc.Bacc("TRN2", target_bir_lowering=False, debug=False)
    x_d = nc.dram_tensor("xq", [SQ, E], F32, kind="ExternalInput")
    out_d = nc.dram_tensor("out", [SQ, E], F32, kind="ExternalOutput")
    with tile.TileContext(nc) as tc, ExitStack() as ctx:
        _build_fast_body(ctx, tc, gamma, x_d, out_d)
    nc.compile()
    return nc


# --------------------------------------------------------------------------
# host-side certification of the block-diagonal mask (exact bounds)
# --------------------------------------------------------------------------

def _screen_fast_ok(x: np.ndarray, gamma: float) -> bool:
    """True iff the fast path is certified correct for this input.

    Guards (g = gamma):
      (a) g > 0 and fp8/bf16 rounding slop of the device logits stays
          < ~55 in exp-argument units (no f32 overflow, no weight
          distortion the margins below can't absorb),
      (b) exact within-block off-diagonal dist2 >= 120/g (threshold 40
          for negligibility + slop headroom for the fp8 gram),
      (c) projected (certified lower-bound) cross-block dist2 >= 31/g.
    """
    g = float(gamma)
    if not np.isfinite(g) or g <= 0.0:
        return False
    Bx, Sx, Ex = x.shape
    sq = np.einsum('bse,bse->bs', x, x).astype(np.float64)
    smax = float(sq.max())
    # device-side exp-argument slop: fp8 gram err tail ~0.012*smax,
    # bf16 sq-row err ~0.001*smax -> arg err <~ 2g*0.013*smax
    if 0.026 * g * smax > 55.0:
        return False
    # (b) exact within-block pairwise distances
    nblk = Sx // P
    xb = np.ascontiguousarray(x.reshape(Bx, nblk, P, Ex))
    Gb = np.einsum('bkie,bkje->bkij', xb, xb)
    sb = sq.reshape(Bx, nblk, P)
    d2b = sb[:, :, :, None] + sb[:, :, None, :] - 2.0 * Gb
    ii = np.arange(P)
    d2b[:, :, ii, ii] = np.inf
    if float(d2b.min()) < 120.0 / g:
        return False
    # (c) cross-block: dist2 >= ||proj(x_i) - proj(x_j)||^2 for
    # orthonormal projection columns (exact bound); 128 dims
    rng = np.random.default_rng(0xA55E77)
    U, _ = np.linalg.qr(rng.standard_normal((Ex, 128)))
    U = np.ascontiguousarray(U.astype(np.float32))
    for b in range(Bx):
        y = x[b] @ U
        sy = np.einsum('se,se->s', y, y)
        Gy = y @ y.T
        d2 = sy[:, None] + sy[None, :] - 2.0 * Gy
        v = d2.reshape(nblk, P, nblk, P)
        for k in range(nblk):
            v[k, :, k, :] = np.inf
        # +1.0 absorbs f32 rounding in this host-side bound computation
        if float(d2.min()) < 31.0 / g + 1.0:
            return False
    return True


# --------------------------------------------------------------------------
# dense fallback (original kernel, unchanged)
# --------------------------------------------------------------------------

def _build_dense_body(ctx: ExitStack, tc: tile.TileContext, gamma: float,
                      xs_d, xo_d, out_d, sqq_d):
    nc = tc.nc

    const = ctx.enter_context(tc.tile_pool(name="const", bufs=1))
    stage = ctx.enter_context(tc.tile_pool(name="stage", bufs=4))
    tpool = ctx.enter_context(tc.tile_pool(name="tpool", bufs=3))
    opool = ctx.enter_context(tc.tile_pool(name="opool", bufs=2))
    small = ctx.enter_context(tc.tile_pool(name="small", bufs=2))
    ptp = ctx.enter_context(tc.tile_pool(name="ptp", bufs=1))

    # ---- persistent SBUF tiles ----
    if FP8_QK:
        # [256-e-chunk][e_part, pair, k]; logical e = 256*c + 128*i + p
        xT8 = [const.tile([P, 2, S], FP8, name=f"xT8{c}", tag=f"xT8{c}")
               for c in range(NEC // 2)]
    else:
        xT = [const.tile([P, S], BF16, name=f"xT{e}", tag=f"xT{e}")
              for e in range(NEC)]                   # [E-chunk][e_part, k]
    V = [const.tile([P, E], BF16, name=f"V{kb}", tag=f"V{kb}")
         for kb in range(NKB)]                       # [k-block][k_part, e]
    sq_all = const.tile([P, NKB], F32, name="sq_all", tag="sq_all")
    biasK = const.tile([P, NKB], F32, name="biasK", tag="biasK")
    sqq_sc = const.tile([P, NKB_SELF], BF16, name="sqq_sc", tag="sqq_sc")
    bcastQ = const.tile([P, SQ], BF16, name="bcastQ", tag="bcastQ")
    ones = const.tile([P, 1], BF16, name="ones", tag="ones")
    nc.vector.memset(ones, 1.0)
    ident = const.tile([P, P], BF16, name="ident", tag="ident")
    make_identity(nc, ident)

    # ---- prologue: load x, compute ||x||^2, cast to bf16, build x^T ----
    with tc.tile_pool(name="sq_ps", bufs=2, space="PSUM") as sq_ps, \
         tc.tile_pool(name="tr_ps", bufs=4, space="PSUM") as tr_ps:
        for kb in range(NKB):
            src = xs_d if kb < NKB_SELF else xo_d
            r0 = (kb % NKB_SELF) * P
            xst = stage.tile([P, E], F32, name="xst", tag="xst")
            nc.sync.dma_start(out=xst, in_=src[r0:r0 + P, :])
            nc.gpsimd.tensor_copy(V[kb], xst)        # f32 -> bf16 cast
            sqt = sq_ps.tile([P, E], F32, name="sqt", tag="sqt")
            nc.scalar.activation(sqt, xst, AF.Square,
                                 accum_out=sq_all[:, kb:kb + 1])
            if FP8_QK:
                for c in range(NEC // 2):
                    trp = tr_ps.tile([P, 2 * P], BF16, name="trp", tag="trp")
                    for i in range(2):
                        nc.tensor.transpose(
                            trp[:, i * P:(i + 1) * P],
                            V[kb][:, (2 * c + i) * P:(2 * c + i + 1) * P],
                            ident)
                    nc.vector.tensor_copy(
                        xT8[c][:, :, kb * P:(kb + 1) * P],
                        trp.rearrange("p (i k) -> p i k", i=2))
            else:
                for e in range(NEC):
                    trp = tr_ps.tile([P, P], BF16, name="trp", tag="trp")
                    nc.tensor.transpose(trp, V[kb][:, e * P:(e + 1) * P],
                                        ident)
                    nc.vector.tensor_copy(xT[e][:, kb * P:(kb + 1) * P], trp)
            if kb == NKB_SELF - 1:
                # self-half stats ready: unblock exp biases + bcastQ early
                nc.vector.tensor_scalar_mul(
                    biasK[:, :NKB_SELF], sq_all[:, :NKB_SELF], -gamma)
                nc.vector.tensor_scalar_mul(
                    sqq_sc, sq_all[:, :NKB_SELF], -0.5)
                nc.sync.dma_start(
                    out=sqq_d[:].rearrange("(c p) -> p c", p=P), in_=sqq_sc)
                s_ap = sqq_d[:]
                bq_src = bass.AP(tensor=s_ap.tensor, offset=s_ap.offset,
                                 ap=[[0, P]] + list(s_ap.ap))
                nc.sync.dma_start(out=bcastQ, in_=bq_src)

    nc.vector.tensor_scalar_mul(biasK[:, NKB_SELF:], sq_all[:, NKB_SELF:],
                                -gamma)

    # ---- main loop: PSUM pools (8 banks total: 2 + 4 + 2) ----
    qk_ps = ctx.enter_context(tc.tile_pool(name="qk_ps", bufs=3, space="PSUM"))
    out_ps = ctx.enter_context(tc.tile_pool(name="out_ps", bufs=2, space="PSUM"))
    s_ps = ctx.enter_context(tc.tile_pool(name="s_ps", bufs=1, space="PSUM"))

    for qb in range(NQB):
        q0 = qb * QB
        # Phase A: P^T[k, q0:q0+QB] for all 32 key blocks
        pts = []
        for kb in range(NKB):
            qkp = qk_ps.tile([P, QB], F32, name="qkp", tag="qkp")
            if FP8_QK:
                for c in range(NEC // 2):
                    nc.tensor.matmul(qkp,
                                     lhsT=xT8[c][:, :, kb * P:(kb + 1) * P],
                                     rhs=xT8[c][:, :, q0:q0 + QB],
                                     start=(c == 0), stop=(c == NEC // 2 - 1),
                                     perf_mode=mybir.MatmulPerfMode.DoubleRow)
            else:
                for e in range(NEC):
                    nc.tensor.matmul(qkp,
                                     lhsT=xT[e][:, kb * P:(kb + 1) * P],
                                     rhs=xT[e][:, q0:q0 + QB],
                                     start=(e == 0), stop=(e == NEC - 1))
            tt = tpool.tile([P, QB], F32, name="tt", tag="tt")
            nc.vector.tensor_add(tt, qkp, bcastQ[:, q0:q0 + QB])
            pt = ptp.tile([P, QB], BF16, name=f"pt{kb}", tag=f"pt{kb}")
            nc.scalar.activation(pt, tt, AF.Exp,
                                 bias=biasK[:, kb:kb + 1], scale=2.0 * gamma)
            pts.append(pt)
        # Phase B: out[q, :] = (P^T)^T @ V, row-sum via ones column
        for qs in range(NQS):
            po = out_ps.tile([P, E], F32, name="po", tag="po")
            sp = s_ps.tile([P, 1], F32, name="sp", tag="sp")
            for kb in range(NKB):
                lw = pts[kb][:, qs * P:(qs + 1) * P]
                nc.tensor.matmul(po[:, 0:EH], lhsT=lw, rhs=V[kb][:, 0:EH],
                                 start=(kb == 0), stop=(kb == NKB - 1))
                nc.tensor.matmul(po[:, EH:E], lhsT=lw, rhs=V[kb][:, EH:E],
                                 start=(kb == 0), stop=(kb == NKB - 1))
                nc.tensor.matmul(sp, lhsT=lw, rhs=ones,
                                 start=(kb == 0), stop=(kb == NKB - 1))
            rc = small.tile([P, 1], F32, name="rc", tag="rc")
            nc.vector.reciprocal(rc, sp)
            ot = opool.tile([P, E], F32, name="ot", tag="ot")
            nc.vector.tensor_scalar_mul(ot, po, rc)
            nc.sync.dma_start(out=out_d[q0 + qs * P:q0 + (qs + 1) * P, :],
                              in_=ot)


def build_dense_module(gamma: float):
    nc = bacc.Bacc("TRN2", target_bir_lowering=False, debug=False)
    xs_d = nc.dram_tensor("x_self", [SQ, E], F32, kind="ExternalInput")
    xo_d = nc.dram_tensor("x_other", [SQ, E], F32, kind="ExternalInput")
    out_d = nc.dram_tensor("out", [SQ, E], F32, kind="ExternalOutput")
    sqq_d = nc.dram_tensor("sqq_scratch", [SQ], BF16)
    with tile.TileContext(nc) as tc, ExitStack() as ctx:
        _build_dense_body(ctx, tc, gamma, xs_d, xo_d, out_d, sqq_d)
    nc.compile()
    return nc


_CACHE: dict = {}


def _get_module(gamma: float, kind: str = "dense"):
    key = (kind, gamma)
    if key not in _CACHE:
        _CACHE[key] = (build_fast_module(gamma) if kind == "fast"
                       else build_dense_module(gamma))
    return _CACHE[key]


def kernel(x, gamma):
    x = np.ascontiguousarray(np.asarray(x, dtype=np.float32))
    g = float(np.asarray(gamma))
    if _screen_fast_ok(x, g):
        nc = _get_module(g, "fast")
        in_maps = []
        for c in range(NCORES):
            b, h = divmod(c, 2)
            in_maps.append({"xq": np.ascontiguousarray(
                x[b, h * SQ:(h + 1) * SQ])})
        res = run_bass_kernel_spmd(nc, in_maps, list(range(NCORES))).results
    else:
        nc = _get_module(g, "dense")
        in_maps = []
        for c in range(NCORES):
            b, h = divmod(c, 2)
            xs = np.ascontiguousarray(x[b, h * SQ:(h + 1) * SQ])
            xo = np.ascontiguousarray(x[b, (1 - h) * SQ:(2 - h) * SQ])
            in_maps.append({"x_self": xs, "x_other": xo})
        res = run_bass_kernel_spmd(nc, in_maps, list(range(NCORES))).results
    out = np.empty((B, S, E), np.float32)
    for c in range(NCORES):
        b, h = divmod(c, 2)
        out[b, h * SQ:(h + 1) * SQ] = res[c]["out"]
    return out


if __name__ == "__main__":
    xs = np.random.randn(B, S, E).astype(np.float32)
    o = kernel(xs, np.float32(1.0))
    print("ran", o.shape, o.dtype)
